# revision 17
# baseline (speedup 1.0000x reference)
"""Multi-head attention (causal, interleaved RoPE) on 8 TRN2 NeuronCores.

Sharding: core c = (batch b = c//4, head-group g = c%4). Each core computes
4 heads of one batch fully on-device (QKV proj + RoPE + causal attention +
partial Wo projection); host sums the 4 row-parallel Wo partials per batch.

Per-core pipeline (phases overlap via a dependency wavefront over 512-wide
t-slices; all tensors are per-slice tiles so Tile's dependency tracking
allows attention(qt) to start once proj(qt) is done):
  proj   x^T slices (f32r) x Wq/Wk column blocks -> PSUM; RoPE applied as
         cos/sin products (DVE) + shifted combines (DVE/GpSimd) writing
         bf16 qT / zero-padded kTz0/kTz1 (so S^T runs full K=128 matmuls,
         keeping the PE HAM-unthrottled at 2.4 GHz)
  attn   per (qt, head-pair): S^T = Kz.T @ qT per k-tile -> exp (ScalarE,
         1/8 scale fused, causal column slicing) -> diagonal triu mask
         (GpSimd) -> PV with lhsT=[V|1] (row 64 = softmax sums) -> recip +
         partition_broadcast + normalize into outT (f32r)
  wo     outT chunks x Wo^T -> partial output, interleaved per q-tile
"""
import numpy as np

import concourse.bass as bass
import concourse.mybir as mybir
import concourse.tile as tile
from concourse import bacc
from concourse.bass_utils import run_bass_kernel_spmd

f32 = mybir.dt.float32
f32r = mybir.dt.float32r
bf16 = mybir.dt.bfloat16
AF = mybir.ActivationFunctionType

T, D = 2048, 1024
G = 4            # heads per core
NTS = 4          # t-slices of 512
TS = T // NTS    # 512
KT = T // 128    # 16 key tiles
DCH = D // 128   # 8 contraction chunks
ROPE_BASE = 10000.0

_CACHE = {}


def _build():
    nc = bacc.Bacc(None, target_bir_lowering=False)
    xt = nc.dram_tensor("xt", [D, T], f32, kind="ExternalInput")
    wqt = nc.dram_tensor("wqt", [D, 256], f32, kind="ExternalInput")
    wkt = nc.dram_tensor("wkt", [D, 256], f32, kind="ExternalInput")
    wvt = nc.dram_tensor("wvt", [D, 256], f32, kind="ExternalInput")
    wot = nc.dram_tensor("wot", [256, D], f32, kind="ExternalInput")
    cosp = nc.dram_tensor("cosp", [128, T], f32, kind="ExternalInput")
    sinp = nc.dram_tensor("sinp", [128, T], f32, kind="ExternalInput")
    triu = nc.dram_tensor("triu", [128, 128], f32, kind="ExternalInput")
    outp = nc.dram_tensor("outp", [T, D], f32, kind="ExternalOutput")

    xt_r = xt.rearrange("(dc p) t -> p dc t", p=128)
    wqt_r = wqt.rearrange("(dc p) j -> p dc j", p=128)
    wkt_r = wkt.rearrange("(dc p) j -> p dc j", p=128)
    wvt_r = wvt.rearrange("(dc p) j -> p dc j", p=128)
    wot_r = wot.rearrange("(c p) m -> p c m", p=128)
    outp_r = outp.rearrange("(tt p) m -> p tt m", p=128)

    with tile.TileContext(nc) as tc:
        with (
            tc.tile_pool(name="const", bufs=1) as const,
            tc.tile_pool(name="xtp", bufs=2) as xtp,
            tc.tile_pool(name="ut", bufs=2) as ut,
            tc.tile_pool(name="expp", bufs=3) as expp,
            tc.tile_pool(name="nrm", bufs=2) as nrm,
            tc.tile_pool(name="osb", bufs=2) as osb,
        ):
            wq_sb = const.tile([128, DCH, 256], f32r)
            wk_sb = const.tile([128, DCH, 256], f32r)
            wv_sb = const.tile([128, DCH, 256], f32r)
            wo_sb = const.tile([128, 2, D], f32r)
            triu_sb = const.tile([128, 128], f32r)
            cos_sb = const.tile([128, T], f32)
            sin_sb = const.tile([128, T], f32)
            nc.sync.dma_start(wq_sb[:], wqt_r.bitcast(f32r))
            nc.sync.dma_start(wk_sb[:], wkt_r.bitcast(f32r))
            nc.sync.dma_start(wv_sb[:], wvt_r.bitcast(f32r))
            nc.sync.dma_start(wo_sb[:], wot_r.bitcast(f32r))
            nc.sync.dma_start(triu_sb[:], triu[:].bitcast(f32r))
            nc.sync.dma_start(cos_sb[:], cosp[:])
            nc.sync.dma_start(sin_sb[:], sinp[:])

            # per-slice tensors (named so dependency tracking stays per-slice)
            qTs = [const.tile([128, 2, TS], bf16, name=f"qT{i}", tag=f"qT{i}") for i in range(NTS)]
            kzs = [
                [const.tile([128, 2, TS], bf16, name=f"kz{hh}_{i}", tag=f"kz{hh}_{i}") for i in range(NTS)]
                for hh in (0, 1)
            ]
            vss = [const.tile([128, 4, G, 65], f32r, name=f"v{i}", tag=f"v{i}") for i in range(NTS)]
            oTs = [const.tile([128, 2, TS], f32r, name=f"oT{i}", tag=f"oT{i}") for i in range(NTS)]
            for hh in (0, 1):
                for i in range(NTS):
                    nc.vector.memset(kzs[hh][i][:], 0.0)
            for i in range(NTS):
                nc.vector.memset(vss[i][:, :, :, 64:65].bitcast(f32), 1.0)

            def proj(tsi, pp, vps, do_v=True, do_qk=True, xt_keep={}):
                sl = slice(tsi * TS, (tsi + 1) * TS)
                xt_t = xtp.tile([128, DCH, TS], f32r, tag="xt", name="xt_t")
                nc.sync.dma_start(xt_t[:], xt_r[:, :, sl].bitcast(f32r))
                for w_sb, is_q in ((wq_sb, True), (wk_sb, False)) if do_qk else ():
                    ps = pp.tile([128, 2, TS], f32, tag="ps", name="ps_qk")
                    for role in (0, 1):
                        for d in range(DCH):
                            nc.tensor.matmul(
                                ps[:, role, :],
                                w_sb[:, d, role * 128:(role + 1) * 128],
                                xt_t[:, d, :],
                                start=(d == 0),
                                stop=(d == DCH - 1),
                            )
                    uc = ut.tile([128, 2, TS], bf16, tag="uc", name="uc")
                    us = ut.tile([128, 2, TS], bf16, tag="us", name="us")
                    nc.vector.tensor_mul(
                        uc[:], ps[:], cos_sb[:, None, sl].to_broadcast((128, 2, TS))
                    )
                    nc.vector.tensor_mul(
                        us[:], ps[:, ::-1, :], sin_sb[:, None, sl].to_broadcast((128, 2, TS))
                    )
                    # combine + relayout to row hh*64 + role*32 + f, chunk hp
                    # (head h = 2*hp + hh); K goes to the hh-th zero-padded tile
                    for h in range(G):
                        hp, hh = h // 2, h % 2
                        src = slice(h * 32, (h + 1) * 32)
                        d2 = qTs[tsi] if is_q else kzs[hh][tsi]
                        eng = nc.vector if is_q else nc.gpsimd
                        eng.tensor_sub(
                            d2[hh * 64:hh * 64 + 32, hp, :], uc[src, 0, :], us[src, 0, :]
                        )
                        eng.tensor_add(
                            d2[hh * 64 + 32:(hh + 1) * 64, hp, :], uc[src, 1, :], us[src, 1, :]
                        )
                if not do_v:
                    return
                for st in range(4):
                    psv = vps.tile([128, 256], f32, tag="v", name="ps_v")
                    for d in range(DCH):
                        nc.tensor.matmul(
                            psv[:],
                            xt_t[:, d, st * 128:(st + 1) * 128],
                            wv_sb[:, d, :],
                            start=(d == 0),
                            stop=(d == DCH - 1),
                        )
                    nc.scalar.copy(
                        vss[tsi][:, st, :, 0:64],
                        psv[:].rearrange("p (g dh) -> p g dh", g=G),
                    )

            def attn(qt, sps, pvp, expp):
                komax = 4 * qt + 3
                pv = [
                    pvp.tile([65, TS], f32, tag=f"pv{h}", name=f"pv{h}")
                    for h in range(G)
                ]
                for ko in range(komax + 1):
                    off = max(0, ko - 4 * qt) * 128
                    tko, kin = divmod(ko, 4)
                    for pair in (0, 1):
                        ps_s = sps.tile([128, 2, TS], f32, tag="s", name="ps_s")
                        for hh in (0, 1):
                            nc.tensor.matmul(
                                ps_s[:, hh, off:],
                                kzs[hh][tko][:, pair, kin * 128:(kin + 1) * 128],
                                qTs[qt][:, pair, off:],
                                start=True,
                                stop=True,
                            )
                        ex = expp.tile([128, 2, TS], f32r, tag="ex", name="ex")
                        nc.scalar.activation(
                            ex[:, :, off:], ps_s[:, :, off:], AF.Exp, scale=0.125
                        )
                        if ko >= 4 * qt:
                            nc.gpsimd.tensor_mul(
                                ex[:, :, off:off + 128],
                                ex[:, :, off:off + 128],
                                triu_sb[:, None, :].to_broadcast((128, 2, 128)),
                            )
                        for hh in (0, 1):
                            nc.tensor.matmul(
                                pv[2 * pair + hh][:, off:],
                                vss[tko][:, kin, 2 * pair + hh, :],
                                ex[:, hh, off:],
                                start=(ko == 0),
                                stop=(ko == komax),
                            )
                for h in range(G):
                    pair, hh = h // 2, h % 2
                    s0 = nrm.tile([1, TS], f32, tag="s0", name="s0")
                    nc.vector.tensor_copy(s0[:], pv[h][64:65, :])
                    rc = nrm.tile([1, TS], f32, tag="rc", name="rc")
                    nc.vector.reciprocal_approx_fast(out=rc[:], in_=s0[:])
                    rb = nrm.tile([64, TS], f32, tag="rb", name="rb")
                    nc.gpsimd.partition_broadcast(rb[:], rc[:])
                    nc.vector.tensor_mul(
                        oTs[qt][hh * 64:(hh + 1) * 64, pair, :], pv[h][0:64, :], rb[:]
                    )

            def wo(qt, wop):
                for t4 in range(4):
                    tt = qt * 4 + t4
                    for mh in (0, 1):
                        po = wop.tile([128, TS], f32, tag="po", name="po")
                        for hc in (0, 1):
                            nc.tensor.matmul(
                                po[:],
                                oTs[qt][:, hc, t4 * 128:(t4 + 1) * 128],
                                wo_sb[:, hc, mh * TS:(mh + 1) * TS],
                                start=(hc == 0),
                                stop=(hc == 1),
                            )
                        ob = osb.tile([128, TS], f32, tag="ob", name="ob")
                        if (tt + mh) % 2 == 0:
                            nc.scalar.copy(ob[:], po[:])
                        else:
                            nc.vector.tensor_copy(ob[:], po[:])
                        nc.sync.dma_start(outp_r[:, tt, mh * TS:(mh + 1) * TS], ob[:])

            # QK projections (V0 with tsi=0); V1-3 at the end as dense PE
            # ballast through the attention-start transition
            xt_keep = {}
            with (
                tc.tile_pool(name="pp", bufs=3, space="PSUM") as pp,
                tc.tile_pool(name="vps", bufs=2, space="PSUM") as vps,
            ):
                for tsi in range(NTS):
                    proj(tsi, pp, vps, xt_keep=xt_keep)
            with (
                tc.tile_pool(name="sps", bufs=2, space="PSUM") as sps,
                tc.tile_pool(name="pvp", bufs=1, space="PSUM") as pvp,
            ):
                for qt in range(NTS):
                    attn(qt, sps, pvp, expp)
            with tc.tile_pool(name="wop", bufs=4, space="PSUM") as wop:
                for qt in range(NTS):
                    wo(qt, wop)
    nc.compile()
    return nc


def _get_nc():
    if "nc" not in _CACHE:
        _CACHE["nc"] = _build()
    return _CACHE["nc"]


def _host_inputs(x, Wq, Wk, Wv, Wo):
    """Build per-core input dicts (host-side sharding / layout prep)."""
    jj = np.arange(256)
    role = jj // 128
    h = (jj % 128) // 32
    f = jj % 32
    inv_freq = 1.0 / (ROPE_BASE ** (np.arange(0, 64, 2, dtype=np.float64) / 64.0))
    t = np.arange(T, dtype=np.float64)
    ang = t[None, :] * inv_freq[np.arange(128) % 32][:, None]   # [128, T]
    cosp = np.cos(ang).astype(np.float32)
    sinp = np.sin(ang).astype(np.float32)
    triu = (np.arange(128)[None, :] >= np.arange(128)[:, None]).astype(np.float32)

    in_maps = []
    for core in range(8):
        b, g = divmod(core, 4)
        jsel = (g * 4 + h) * 64 + 2 * f + role
        in_maps.append({
            "xt": np.ascontiguousarray(x[b].T),
            "wqt": np.ascontiguousarray(Wq[jsel, :].T),
            "wkt": np.ascontiguousarray(Wk[jsel, :].T),
            "wvt": np.ascontiguousarray(Wv[g * 256:(g + 1) * 256, :].T),
            "wot": np.ascontiguousarray(Wo[:, g * 256:(g + 1) * 256].T),
            "cosp": cosp,
            "sinp": sinp,
            "triu": triu,
        })
    return in_maps


def run(x, Wq, Wk, Wv, Wo, trace=False):
    nc = _get_nc()
    in_maps = _host_inputs(x, Wq, Wk, Wv, Wo)
    res = run_bass_kernel_spmd(nc, in_maps, core_ids=list(range(8)), trace=trace)
    out = np.zeros((2, T, D), dtype=np.float64)
    for core in range(8):
        out[core // 4] += res.results[core]["outp"].astype(np.float64)
    return out.astype(np.float32), res


def kernel(x=None, mask=None, Wq=None, Wk=None, Wv=None, Wo=None, **_ignored):
    x = np.asarray(x, dtype=np.float32)
    Wq = np.asarray(Wq, dtype=np.float32)
    Wk = np.asarray(Wk, dtype=np.float32)
    Wv = np.asarray(Wv, dtype=np.float32)
    Wo = np.asarray(Wo, dtype=np.float32)
    out, _ = run(x, Wq, Wk, Wv, Wo, trace=False)
    return out


# revision 18
# speedup vs baseline: 1.1324x; 1.1324x over previous
"""Multi-head attention (causal, interleaved RoPE) on 8 TRN2 NeuronCores.

Sharding: core c = (batch b = c//4, head-group g = c%4). Each core computes
4 heads of one batch fully on-device (QKV proj + RoPE + causal attention +
partial Wo projection); host sums the 4 row-parallel Wo partials per batch.

Per-core pipeline (phases overlap via a dependency wavefront over 512-wide
t-slices; all tensors are per-slice tiles so Tile's dependency tracking
allows attention(qt) to start once proj(qt) is done):
  proj   x^T slices (f32r) x Wq/Wk column blocks -> PSUM; RoPE applied as
         cos/sin products (DVE) + shifted combines (DVE/GpSimd) writing
         bf16 qT / zero-padded kTz0/kTz1 (so S^T runs full K=128 matmuls,
         keeping the PE HAM-unthrottled at 2.4 GHz)
  attn   per (qt, head-pair): S^T = Kz.T @ qT per k-tile -> exp (ScalarE,
         1/8 scale fused, causal column slicing) -> diagonal triu mask
         (GpSimd) -> PV with lhsT=[V|1] (row 64 = softmax sums) -> recip +
         partition_broadcast + normalize into outT (f32r)
  wo     outT chunks x Wo^T -> partial output, interleaved per q-tile
"""
import numpy as np

import concourse.bass as bass
import concourse.mybir as mybir
import concourse.tile as tile
from concourse import bacc
from concourse.bass_utils import run_bass_kernel_spmd

f32 = mybir.dt.float32
f32r = mybir.dt.float32r
bf16 = mybir.dt.bfloat16
AF = mybir.ActivationFunctionType

T, D = 2048, 1024
G = 4            # heads per core
NTS = 4          # t-slices of 512
TS = T // NTS    # 512
KT = T // 128    # 16 key tiles
DCH = D // 128   # 8 contraction chunks
ROPE_BASE = 10000.0

_CACHE = {}


def _build():
    nc = bacc.Bacc(None, target_bir_lowering=False)
    xt = nc.dram_tensor("xt", [D, T], f32, kind="ExternalInput")
    wqt = nc.dram_tensor("wqt", [D, 256], f32, kind="ExternalInput")
    wkt = nc.dram_tensor("wkt", [D, 256], f32, kind="ExternalInput")
    wvt = nc.dram_tensor("wvt", [D, 256], f32, kind="ExternalInput")
    wot = nc.dram_tensor("wot", [256, D], f32, kind="ExternalInput")
    cosp = nc.dram_tensor("cosp", [128, T], f32, kind="ExternalInput")
    sinp = nc.dram_tensor("sinp", [128, T], f32, kind="ExternalInput")
    triu = nc.dram_tensor("triu", [128, 128], f32, kind="ExternalInput")
    outp = nc.dram_tensor("outp", [T, D], f32, kind="ExternalOutput")

    xt_r = xt.rearrange("(dc p) t -> p dc t", p=128)
    wqt_r = wqt.rearrange("(dc p) j -> p dc j", p=128)
    wkt_r = wkt.rearrange("(dc p) j -> p dc j", p=128)
    wvt_r = wvt.rearrange("(dc p) j -> p dc j", p=128)
    wot_r = wot.rearrange("(c p) m -> p c m", p=128)
    outp_r = outp.rearrange("(tt p) m -> p tt m", p=128)

    with tile.TileContext(nc) as tc:
        with (
            tc.tile_pool(name="const", bufs=1) as const,
            tc.tile_pool(name="xtp", bufs=2) as xtp,
            tc.tile_pool(name="ut", bufs=2) as ut,
            tc.tile_pool(name="expp", bufs=3) as expp,
            tc.tile_pool(name="nrm", bufs=2) as nrm,
            tc.tile_pool(name="osb", bufs=2) as osb,
        ):
            wq_sb = const.tile([128, DCH, 256], f32r)
            wk_sb = const.tile([128, DCH, 256], f32r)
            wv_sb = const.tile([128, DCH, 256], f32r)
            wo_sb = const.tile([128, 2, D], f32r)
            triu_sb = const.tile([128, 128], f32r)
            cos_sb = const.tile([128, T], f32)
            sin_sb = const.tile([128, T], f32)
            nc.sync.dma_start(wq_sb[:], wqt_r.bitcast(f32r))
            nc.sync.dma_start(wk_sb[:], wkt_r.bitcast(f32r))
            nc.sync.dma_start(wv_sb[:], wvt_r.bitcast(f32r))
            nc.sync.dma_start(wo_sb[:], wot_r.bitcast(f32r))
            nc.sync.dma_start(triu_sb[:], triu[:].bitcast(f32r))
            nc.sync.dma_start(cos_sb[:], cosp[:])
            nc.sync.dma_start(sin_sb[:], sinp[:])

            # per-slice tensors (named so dependency tracking stays per-slice)
            qTs = [const.tile([128, 2, TS], bf16, name=f"qT{i}", tag=f"qT{i}") for i in range(NTS)]
            kzs = [
                [const.tile([128, 2, TS], bf16, name=f"kz{hh}_{i}", tag=f"kz{hh}_{i}") for i in range(NTS)]
                for hh in (0, 1)
            ]
            vss = [const.tile([128, 4, G, 65], f32r, name=f"v{i}", tag=f"v{i}") for i in range(NTS)]
            oTs = [const.tile([128, 2, TS], f32r, name=f"oT{i}", tag=f"oT{i}") for i in range(NTS)]
            for hh in (0, 1):
                for i in range(NTS):
                    nc.vector.memset(kzs[hh][i][:], 0.0)
            for i in range(NTS):
                nc.vector.memset(vss[i][:, :, :, 64:65].bitcast(f32), 1.0)

            def proj(tsi, pp, vps, do_v=True, do_qk=True, xt_keep={}):
                sl = slice(tsi * TS, (tsi + 1) * TS)
                xt_t = xtp.tile([128, DCH, TS], f32r, tag="xt", name="xt_t")
                nc.sync.dma_start(xt_t[:], xt_r[:, :, sl].bitcast(f32r))
                for w_sb, is_q in ((wq_sb, True), (wk_sb, False)) if do_qk else ():
                    ps = pp.tile([128, 2, TS], f32, tag="ps", name="ps_qk")
                    for role in (0, 1):
                        for d in range(DCH):
                            nc.tensor.matmul(
                                ps[:, role, :],
                                w_sb[:, d, role * 128:(role + 1) * 128],
                                xt_t[:, d, :],
                                start=(d == 0),
                                stop=(d == DCH - 1),
                            )
                    uc = ut.tile([128, 2, TS], bf16, tag="uc", name="uc")
                    us = ut.tile([128, 2, TS], bf16, tag="us", name="us")
                    nc.vector.tensor_mul(
                        uc[:], ps[:], cos_sb[:, None, sl].to_broadcast((128, 2, TS))
                    )
                    nc.vector.tensor_mul(
                        us[:], ps[:, ::-1, :], sin_sb[:, None, sl].to_broadcast((128, 2, TS))
                    )
                    # combine + relayout to row hh*64 + role*32 + f, chunk hp
                    # (head h = 2*hp + hh); K goes to the hh-th zero-padded tile
                    for h in range(G):
                        hp, hh = h // 2, h % 2
                        src = slice(h * 32, (h + 1) * 32)
                        d2 = qTs[tsi] if is_q else kzs[hh][tsi]
                        eng = nc.vector if is_q else nc.gpsimd
                        eng.tensor_sub(
                            d2[hh * 64:hh * 64 + 32, hp, :], uc[src, 0, :], us[src, 0, :]
                        )
                        eng.tensor_add(
                            d2[hh * 64 + 32:(hh + 1) * 64, hp, :], uc[src, 1, :], us[src, 1, :]
                        )
                if not do_v:
                    return
                for st in range(4):
                    psv = vps.tile([128, 256], f32, tag="v", name="ps_v")
                    for d in range(DCH):
                        nc.tensor.matmul(
                            psv[:],
                            xt_t[:, d, st * 128:(st + 1) * 128],
                            wv_sb[:, d, :],
                            start=(d == 0),
                            stop=(d == DCH - 1),
                        )
                    nc.scalar.copy(
                        vss[tsi][:, st, :, 0:64],
                        psv[:].rearrange("p (g dh) -> p g dh", g=G),
                    )

            def attn(qt, sps, pvp, expp):
                komax = 4 * qt + 3
                pv = [
                    pvp.tile([65, TS], f32, tag=f"pv{h}", name=f"pv{h}")
                    for h in range(G)
                ]
                for ko in range(komax + 1):
                    off = max(0, ko - 4 * qt) * 128
                    tko, kin = divmod(ko, 4)
                    for pair in (0, 1):
                        ps_s = sps.tile([128, 2, TS], f32, tag="s", name="ps_s")
                        for hh in (0, 1):
                            nc.tensor.matmul(
                                ps_s[:, hh, off:],
                                kzs[hh][tko][:, pair, kin * 128:(kin + 1) * 128],
                                qTs[qt][:, pair, off:],
                                start=True,
                                stop=True,
                            )
                        ex = expp.tile([128, 2, TS], f32r, tag="ex", name="ex")
                        nc.scalar.activation(
                            ex[:, :, off:], ps_s[:, :, off:], AF.Exp, scale=0.125
                        )
                        if ko >= 4 * qt:
                            nc.vector.tensor_mul(
                                ex[:, :, off:off + 128],
                                ex[:, :, off:off + 128],
                                triu_sb[:, None, :].to_broadcast((128, 2, 128)),
                            )
                        for hh in (0, 1):
                            nc.tensor.matmul(
                                pv[2 * pair + hh][:, off:],
                                vss[tko][:, kin, 2 * pair + hh, :],
                                ex[:, hh, off:],
                                start=(ko == 0),
                                stop=(ko == komax),
                            )
                for h in range(G):
                    pair, hh = h // 2, h % 2
                    s0 = nrm.tile([1, TS], f32, tag="s0", name="s0")
                    nc.vector.tensor_copy(s0[:], pv[h][64:65, :])
                    rc = nrm.tile([1, TS], f32, tag="rc", name="rc")
                    nc.vector.reciprocal_approx_fast(out=rc[:], in_=s0[:])
                    rb = nrm.tile([64, TS], f32, tag="rb", name="rb")
                    nc.gpsimd.partition_broadcast(rb[:], rc[:])
                    nc.vector.tensor_mul(
                        oTs[qt][hh * 64:(hh + 1) * 64, pair, :], pv[h][0:64, :], rb[:]
                    )

            def wo(qt, wop):
                for t4 in range(4):
                    tt = qt * 4 + t4
                    for mh in (0, 1):
                        po = wop.tile([128, TS], f32, tag="po", name="po")
                        for hc in (0, 1):
                            nc.tensor.matmul(
                                po[:],
                                oTs[qt][:, hc, t4 * 128:(t4 + 1) * 128],
                                wo_sb[:, hc, mh * TS:(mh + 1) * TS],
                                start=(hc == 0),
                                stop=(hc == 1),
                            )
                        ob = osb.tile([128, TS], f32, tag="ob", name="ob")
                        if (tt + mh) % 2 == 0:
                            nc.scalar.copy(ob[:], po[:])
                        else:
                            nc.vector.tensor_copy(ob[:], po[:])
                        nc.sync.dma_start(outp_r[:, tt, mh * TS:(mh + 1) * TS], ob[:])

            # QK projections (V0 with tsi=0); V1-3 at the end as dense PE
            # ballast through the attention-start transition
            xt_keep = {}
            with (
                tc.tile_pool(name="pp", bufs=3, space="PSUM") as pp,
                tc.tile_pool(name="vps", bufs=2, space="PSUM") as vps,
            ):
                for tsi in range(NTS):
                    proj(tsi, pp, vps, xt_keep=xt_keep)
            with (
                tc.tile_pool(name="sps", bufs=2, space="PSUM") as sps,
                tc.tile_pool(name="pvp", bufs=1, space="PSUM") as pvp,
            ):
                for qt in range(NTS):
                    attn(qt, sps, pvp, expp)
            with tc.tile_pool(name="wop", bufs=4, space="PSUM") as wop:
                for qt in range(NTS):
                    wo(qt, wop)
    nc.compile()
    return nc


def _get_nc():
    if "nc" not in _CACHE:
        _CACHE["nc"] = _build()
    return _CACHE["nc"]


def _host_inputs(x, Wq, Wk, Wv, Wo):
    """Build per-core input dicts (host-side sharding / layout prep)."""
    jj = np.arange(256)
    role = jj // 128
    h = (jj % 128) // 32
    f = jj % 32
    inv_freq = 1.0 / (ROPE_BASE ** (np.arange(0, 64, 2, dtype=np.float64) / 64.0))
    t = np.arange(T, dtype=np.float64)
    ang = t[None, :] * inv_freq[np.arange(128) % 32][:, None]   # [128, T]
    cosp = np.cos(ang).astype(np.float32)
    sinp = np.sin(ang).astype(np.float32)
    triu = (np.arange(128)[None, :] >= np.arange(128)[:, None]).astype(np.float32)

    in_maps = []
    for core in range(8):
        b, g = divmod(core, 4)
        jsel = (g * 4 + h) * 64 + 2 * f + role
        in_maps.append({
            "xt": np.ascontiguousarray(x[b].T),
            "wqt": np.ascontiguousarray(Wq[jsel, :].T),
            "wkt": np.ascontiguousarray(Wk[jsel, :].T),
            "wvt": np.ascontiguousarray(Wv[g * 256:(g + 1) * 256, :].T),
            "wot": np.ascontiguousarray(Wo[:, g * 256:(g + 1) * 256].T),
            "cosp": cosp,
            "sinp": sinp,
            "triu": triu,
        })
    return in_maps


def run(x, Wq, Wk, Wv, Wo, trace=False):
    nc = _get_nc()
    in_maps = _host_inputs(x, Wq, Wk, Wv, Wo)
    res = run_bass_kernel_spmd(nc, in_maps, core_ids=list(range(8)), trace=trace)
    out = np.zeros((2, T, D), dtype=np.float64)
    for core in range(8):
        out[core // 4] += res.results[core]["outp"].astype(np.float64)
    return out.astype(np.float32), res


def kernel(x=None, mask=None, Wq=None, Wk=None, Wv=None, Wo=None, **_ignored):
    x = np.asarray(x, dtype=np.float32)
    Wq = np.asarray(Wq, dtype=np.float32)
    Wk = np.asarray(Wk, dtype=np.float32)
    Wv = np.asarray(Wv, dtype=np.float32)
    Wo = np.asarray(Wo, dtype=np.float32)
    out, _ = run(x, Wq, Wk, Wv, Wo, trace=False)
    return out
